# revision 56
# baseline (speedup 1.0000x reference)
"""Trainium2 Bass kernel for CapsuleLayer dynamic routing (B=128, I=1152, J=128, K=32, D=32).

Strategy
--------
Data-parallel over batch: 16 samples per core x 8 cores. Routing is
algebraically factorized so u_hat [B,I,K,D] (604 MB) is never materialized:

    y[s,k,j]  = sum_i c[s,i,k] x[s,i,j]       (per-sample PE matmul, i contracted)
    sT[d,k,s] = sum_j W[j,k,d] y[s,k,j]       (col-tiled quad PE matmuls, j contracted)
    v         = squash(sT)                    (ACT/DVE in transposed layout)
    t[j,k,s]  = sum_d W[j,k,d] vT[d,k,s]      (row-tiled quad PE matmuls, d contracted)
    b[s,i,k] += sum_j x[s,i,j] t[s,j,k]       (per-sample PE matmul, j contracted)

b is never materialized: c2 = softmax(b0+b1) uses exp(b0+b1) = exp(b0)*exp(b1)
(exp taken straight off the b-update PSUM by ACT). The squash sum-over-d runs
as a block-diagonal ones matmul on PE, which also replicates ss across each
32-partition group (free partition-broadcast). sqrt is computed as
exp(-0.5*ln(ss+eps)) so ACT stays inside the natural_log_exp table set the
whole kernel (no ACT_TABLE_LOAD switches). x is staged bf16 (i-major) for the
y-matmuls and fp8-e4m3 (j-major) for the agreement path; weights bf16.
"""
import numpy as np
import ml_dtypes
from contextlib import ExitStack

import concourse.bass as bass
import concourse.bacc as bacc_mod
import concourse.mybir as mybir
import concourse.tile as tile
from concourse.bass_utils import run_bass_kernel_spmd
from concourse.masks import make_identity

B, I, J, K, D = 128, 1152, 128, 32, 32
NCORES = 8
S = B // NCORES          # 16 samples per core
CH = I // 128            # 9 chunks of the input-capsule axis
NUM_ROUTING = 3
EPS = 1e-7
F32 = mybir.dt.float32
BF16 = mybir.dt.bfloat16
F8 = mybir.dt.float8e4
AF = mybir.ActivationFunctionType
SG = 4                   # softmax sample-group size

_PROGRAM = None
DEBUG = False
XB_FP8 = True


def _build_program():
    XB_DT = F8 if XB_FP8 else BF16
    nc = bacc_mod.Bacc("TRN2", target_bir_lowering=False, debug=False,
                       num_devices=NCORES)
    y0_d = nc.dram_tensor("y0", [128, S], BF16, kind="ExternalInput")
    wr_d = nc.dram_tensor("wr", [128, K, D], BF16, kind="ExternalInput")
    wt_d = nc.dram_tensor("wt", [128, K // 4, 128], BF16, kind="ExternalInput")
    xb_d = nc.dram_tensor("xb", [128, S, CH * 128], XB_DT, kind="ExternalInput")
    xa_d = nc.dram_tensor("xa", [128, S, CH, 128], BF16, kind="ExternalInput")
    # vout in [(kq, s), (g, d)] layout (k = 4*kq + g); unshuffled host-side
    v_d = nc.dram_tensor("vout", [128, 128], F32, kind="ExternalOutput")
    if DEBUG:
        dbg = {
            "d_sq0": nc.dram_tensor("d_sq0", [128, K // 4, S], BF16, kind="ExternalOutput"),
            "d_sT0": nc.dram_tensor("d_sT0", [128, K // 4, S], F32, kind="ExternalOutput"),
            "d_ss0": nc.dram_tensor("d_ss0", [128, K // 4, S], F32, kind="ExternalOutput"),
            "d_L0": nc.dram_tensor("d_L0", [128, K // 4, S], F32, kind="ExternalOutput"),
            "d_q0": nc.dram_tensor("d_q0", [128, K // 4, S], F32, kind="ExternalOutput"),
            "d_sc0": nc.dram_tensor("d_sc0", [128, K // 4, S], F32, kind="ExternalOutput"),
            "d_vT0": nc.dram_tensor("d_vT0", [128, K // 4, S], BF16, kind="ExternalOutput"),
            "d_T20": nc.dram_tensor("d_T20", [128, S, K], BF16, kind="ExternalOutput"),
            "d_exp0": nc.dram_tensor("d_exp0", [128, S, CH, K], BF16, kind="ExternalOutput"),
            "d_cs1": nc.dram_tensor("d_cs1", [128, S, CH, K], BF16, kind="ExternalOutput"),
            "d_Y21": nc.dram_tensor("d_Y21", [128, S, K], BF16, kind="ExternalOutput"),
            "d_exp1": nc.dram_tensor("d_exp1", [128, S, CH, K], BF16, kind="ExternalOutput"),
        }

    with tile.TileContext(nc) as tc, ExitStack() as ctx, \
            nc.allow_low_precision(reason="bf16 softmax stats validated offline"):
        const = ctx.enter_context(tc.tile_pool(name="const", bufs=1))
        xap = ctx.enter_context(tc.tile_pool(name="xa", bufs=1))
        xbp = ctx.enter_context(tc.tile_pool(name="xb", bufs=1))
        expp = ctx.enter_context(tc.tile_pool(name="exp", bufs=1))
        csp = ctx.enter_context(tc.tile_pool(name="cs", bufs=1))
        zp = ctx.enter_context(tc.tile_pool(name="z", bufs=2))
        sqp = ctx.enter_context(tc.tile_pool(name="sq", bufs=1))
        vtp = ctx.enter_context(tc.tile_pool(name="vt", bufs=1))
        t2p = ctx.enter_context(tc.tile_pool(name="t2", bufs=1))
        y2p = ctx.enter_context(tc.tile_pool(name="y2", bufs=1))
        outp = ctx.enter_context(tc.tile_pool(name="out", bufs=1))
        ps_sT = ctx.enter_context(tc.tile_pool(name="ps_sT", bufs=1, space="PSUM"))
        ps_t = ctx.enter_context(tc.tile_pool(name="ps_t", bufs=1, space="PSUM"))
        ps_y = ctx.enter_context(tc.tile_pool(name="ps_y", bufs=1, space="PSUM"))
        ps_bu = ctx.enter_context(tc.tile_pool(name="ps_bu", bufs=3, space="PSUM"))
        ps_m = ctx.enter_context(tc.tile_pool(name="ps_m", bufs=1, space="PSUM"))

        # ---- input DMAs. Two HWDGE queues (sync, scalar) so descriptor
        # generation parallelizes; FIFO order per queue = priority order.
        y0t = const.tile([128, S], BF16)
        nc.sync.dma_start(out=y0t, in_=y0_d[:])
        wr = const.tile([128, K, D], BF16)
        nc.sync.dma_start(out=wr, in_=wr_d[:])
        wt = const.tile([128, K // 4, 128], BF16)
        nc.scalar.dma_start(out=wt, in_=wt_d[:])
        xb = xbp.tile([128, S, CH * 128], XB_DT)
        xa = xap.tile([128, S, CH, 128], BF16)
        dma_q = [nc.sync, nc.scalar]
        for g in range(S // SG):
            dma_q[g % 2].dma_start(out=xb[:, g * SG:(g + 1) * SG],
                                   in_=xb_d[:, g * SG:(g + 1) * SG])
            dma_q[(g + 1) % 2].dma_start(out=xa[:, g * SG:(g + 1) * SG],
                                         in_=xa_d[:, g * SG:(g + 1) * SG])

        eps128 = const.tile([128, 1], F32)
        nc.vector.memset(eps128, EPS)
        ident = const.tile([128, 128], F32)
        make_identity(nc, ident)

        # block-diagonal ones [128, 128]: ones2[p, m] = 1 iff p//32 == m//32
        ones2 = const.tile([128, 128], BF16)
        nc.vector.memset(ones2, 0.0)
        for g in range(4):
            nc.vector.memset(ones2[32 * g:32 * (g + 1), 32 * g:32 * (g + 1)], 1.0)
        # mask4[p, g] = 1 iff p//32 == g
        mask4 = const.tile([128, 4], BF16)
        nc.vector.memset(mask4, 0.0)
        for g in range(4):
            nc.vector.memset(mask4[32 * g:32 * (g + 1), g:g + 1], 1.0)

        exp0 = expp.tile([128, S, CH, K], BF16, tag="e0")
        exp1 = expp.tile([128, S, CH, K], BF16, tag="e1")

        # ACT table-set preloading: sqrt and exp live in different table
        # sets (~2.7us switch). Tiny dummy activations, data-chained to the
        # producing stage, force each switch to happen while PE streams run.
        dummy = const.tile([1, 1], F32)

        def preload(func, dep_ap):
            nc.scalar.activation(out=dummy, in_=dep_ap, func=func)

        preload(AF.Sqrt, eps128[0:1, :])  # load sqrt set during startup DMA

        def sT_matmuls(rhs_per_k):
            """sT_ps[(k%4)*32+d, k//4, s] = sum_j wr[j,k,d] * rhs_k[j,s]"""
            sT = ps_sT.tile([128, K // 4, S], F32, tag="sT")
            for kq in range(K // 4):
                for g in range(4):
                    k = 4 * kq + g
                    nc.tensor.matmul(sT[32 * g:32 * (g + 1), kq, :],
                                     wr[:, k, :], rhs_per_k(k),
                                     start=True, stop=True,
                                     tile_position=(0, 32 * g))
            return sT

        def squash01(sT, it, out_dt=BF16, do_preload=True):
            """vT[d,k,s] = squash(sT) along d, in the transposed layout.
            Square is a filler ACT func (no table load); Sqrt's table load
            is hidden by the preload dummies."""
            sq = sqp.tile([128, K // 4, S], BF16, tag="sq")
            nc.scalar.activation(out=sq, in_=sT, func=AF.Square)
            ss = ps_m.tile([128, K // 4, S], F32, tag="m")
            nc.tensor.matmul(ss, ones2, sq, start=True, stop=True)
            q = sqp.tile([128, K // 4, S], F32, tag="q")
            nc.scalar.activation(out=q, in_=ss, func=AF.Sqrt, bias=eps128)
            if do_preload:
                preload(AF.Exp, q[0:1, 0, 0:1])  # reload exp set during t/bu
            d1 = sqp.tile([128, K // 4, S], F32, tag="d1")
            nc.vector.scalar_tensor_tensor(out=d1, in0=ss, scalar=1.0, in1=q,
                                           op0=mybir.AluOpType.add,
                                           op1=mybir.AluOpType.mult)
            r1 = sqp.tile([128, K // 4, S], F32, tag="r1")
            nc.vector.reciprocal(out=r1, in_=d1)
            sc = sqp.tile([128, K // 4, S], F32, tag="sc")
            nc.vector.tensor_mul(sc, ss, r1)
            vT = vtp.tile([128, K // 4, S], out_dt, tag="vT" + str(out_dt))
            nc.vector.tensor_mul(vT, sT, sc)
            if DEBUG and it == 0:
                nc.scalar.dma_start(out=dbg["d_q0"][:], in_=q)
                nc.scalar.dma_start(out=dbg["d_sc0"][:], in_=sc)
            return vT

        def t_stage(vT, it):
            """T2[j, s, k] = sum_d W[j,k,d] vT[d,k,s].

            Per k-quad ONE matmul of N=64: rhs vTz[:, kq, g, s] is vT
            masked to partition group g, so contracting all 128 partitions
            picks out k = 4kq+g for output column (g, s)."""
            vTz = vtp.tile([128, K // 4, 4, S], BF16, tag="vTz")
            nc.vector.tensor_tensor(
                out=vTz,
                in0=vT.unsqueeze(2).broadcast_to([128, K // 4, 4, S]),
                in1=mask4.unsqueeze(1).unsqueeze(-1).broadcast_to([128, K // 4, 4, S]),
                op=mybir.AluOpType.mult)
            t_ps = ps_t.tile([128, K // 4, 4, S], F32, tag="t")
            for kq in range(K // 4):
                nc.tensor.matmul(t_ps[:, kq], wt[:, kq, :], vTz[:, kq],
                                 start=True, stop=True)
            T2 = t2p.tile([128, S, K], BF16, tag="T2")
            nc.vector.tensor_copy(
                out=T2.rearrange("p s (q g) -> p q g s", q=K // 4),
                in_=t_ps)
            return T2

        def bu_stage(T2, exp_t):
            """b-update for all samples; exp of each sample's bu -> exp_t."""
            for s in range(S):
                bu = ps_bu.tile([128, CH, K], F32, tag="bu")
                for ic in range(CH):
                    nc.tensor.matmul(bu[:, ic, :],
                                     xb[:, s, ic * 128:(ic + 1) * 128],
                                     T2[:, s, :], start=True, stop=True)
                nc.scalar.activation(out=exp_t[:, s], in_=bu, func=AF.Exp)
            preload(AF.Sqrt, exp_t[0:1, S - 1, 0, 0:1])  # sqrt set during y

        def y_matmuls(cs):
            y_ps = ps_y.tile([128, S, K], F32, tag="y")
            for s in range(S):
                for ic in range(CH):
                    nc.tensor.matmul(y_ps[:, s, :], xa[:, s, ic, :],
                                     cs[:, s, ic, :],
                                     start=(ic == 0), stop=(ic == CH - 1))
            Y2 = y2p.tile([128, S, K], BF16, tag="Y2")
            nc.scalar.activation(out=Y2, in_=y_ps, func=AF.Copy)
            return Y2

        # ================= iteration 0 =================
        sT = sT_matmuls(lambda k: y0t)
        vT = squash01(sT, 0)
        T2 = t_stage(vT, 0)
        bu_stage(T2, exp0)
        if DEBUG:
            nc.scalar.dma_start(out=dbg["d_vT0"][:], in_=vT)
            nc.scalar.dma_start(out=dbg["d_T20"][:], in_=T2)
            nc.scalar.dma_start(out=dbg["d_exp0"][:], in_=exp0)

        # ---- softmax it1: cs1 = exp0 / z
        cs = csp.tile([128, S, CH, K], BF16, tag="cs")
        for g in range(S // SG):
            sl = slice(g * SG, (g + 1) * SG)
            z = zp.tile([128, SG, CH], BF16, tag="z")
            nc.vector.tensor_reduce(out=z, in_=exp0[:, sl],
                                    axis=mybir.AxisListType.X,
                                    op=mybir.AluOpType.add)
            r = zp.tile([128, SG, CH], BF16, tag="r")
            nc.vector.reciprocal(out=r, in_=z)
            nc.vector.tensor_tensor(out=cs[:, sl], in0=exp0[:, sl],
                                    in1=r.unsqueeze(-1).broadcast_to([128, SG, CH, K]),
                                    op=mybir.AluOpType.mult)

        if DEBUG:
            nc.scalar.dma_start(out=dbg["d_cs1"][:], in_=cs)

        # ================= iteration 1 =================
        Y2 = y_matmuls(cs)
        sT = sT_matmuls(lambda k: Y2[:, :, k])
        vT = squash01(sT, 1)
        T2 = t_stage(vT, 1)
        bu_stage(T2, exp1)
        if DEBUG:
            nc.scalar.dma_start(out=dbg["d_Y21"][:], in_=Y2)
            nc.scalar.dma_start(out=dbg["d_exp1"][:], in_=exp1)

        # ---- softmax it2: cs2 = exp0*exp1 / z2
        cs2 = csp.tile([128, S, CH, K], BF16, tag="cs2")
        for g in range(S // SG):
            sl = slice(g * SG, (g + 1) * SG)
            e2 = csp.tile([128, SG, CH, K], BF16, tag="e2")
            nc.vector.tensor_tensor(out=e2, in0=exp0[:, sl], in1=exp1[:, sl],
                                    op=mybir.AluOpType.mult)
            z = zp.tile([128, SG, CH], BF16, tag="z")
            nc.vector.tensor_reduce(out=z, in_=e2, axis=mybir.AxisListType.X,
                                    op=mybir.AluOpType.add)
            r = zp.tile([128, SG, CH], BF16, tag="r")
            nc.vector.reciprocal(out=r, in_=z)
            nc.vector.tensor_tensor(out=cs2[:, sl], in0=e2,
                                    in1=r.unsqueeze(-1).broadcast_to([128, SG, CH, K]),
                                    op=mybir.AluOpType.mult)

        # ================= iteration 2 =================
        Y2 = y_matmuls(cs2)
        sT = sT_matmuls(lambda k: Y2[:, :, k])
        vT = squash01(sT, 2, out_dt=F32, do_preload=False)
        # transpose [(g,d), (kq,s)] -> [(kq,s), (g,d)] and store
        vt_ps = ps_m.tile([128, 128], F32, tag="m")
        nc.tensor.transpose(out=vt_ps, in_=vT.rearrange("p q s -> p (q s)"),
                            identity=ident)
        vsb = outp.tile([128, 128], F32, tag="v")
        nc.scalar.activation(out=vsb, in_=vt_ps, func=AF.Copy)
        nc.sync.dma_start(out=v_d[:], in_=vsb)

    nc.compile()
    return nc


def _get_program():
    global _PROGRAM
    if _PROGRAM is None:
        _PROGRAM = _build_program()
    return _PROGRAM


def _prep_core_inputs(x_core, Wf):
    """x_core: [S, I, J] fp32, Wf: [J, K, D] fp32 -> per-core input map."""
    bf = ml_dtypes.bfloat16
    f8 = ml_dtypes.float8_e4m3 if XB_FP8 else ml_dtypes.bfloat16
    y0 = np.ascontiguousarray((x_core.sum(axis=1, dtype=np.float32) / K).T.astype(bf))
    xa = np.ascontiguousarray(
        x_core.reshape(S, CH, 128, J).transpose(2, 0, 1, 3).astype(bf))
    xb = np.ascontiguousarray(
        x_core.transpose(2, 0, 1).astype(f8)).reshape(J, S, CH * 128)
    wr = np.ascontiguousarray(Wf.astype(bf))
    # wt[(k%4)*32 + d, k//4, j] = W[j, k, d]
    wt = np.ascontiguousarray(
        Wf.transpose(1, 2, 0).reshape(K // 4, 4, D, J)
        .transpose(1, 2, 0, 3).reshape(128, K // 4, J).astype(bf))
    return {"y0": y0, "wr": wr, "wt": wt, "xb": xb, "xa": xa}


def kernel(inputs, W):
    x = np.ascontiguousarray(np.asarray(inputs, dtype=np.float32))
    Wf = np.ascontiguousarray(np.asarray(W, dtype=np.float32))
    nc = _get_program()
    in_maps = [_prep_core_inputs(x[c * S:(c + 1) * S], Wf) for c in range(NCORES)]
    res = run_bass_kernel_spmd(nc, in_maps, list(range(NCORES)))
    return np.concatenate([_unshuffle_v(r["vout"]) for r in res.results], axis=0)


def _unshuffle_v(arr):
    """[(kq, s), (g, d)] fp32 -> [s, k, d] with k = 4*kq + g."""
    return np.ascontiguousarray(
        arr.reshape(K // 4, S, 4, D).transpose(1, 0, 2, 3).reshape(S, K, D))


# revision 57
# speedup vs baseline: 1.0216x; 1.0216x over previous
"""Trainium2 Bass kernel for CapsuleLayer dynamic routing (B=128, I=1152, J=128, K=32, D=32).

Strategy
--------
Data-parallel over batch: 16 samples per core x 8 cores. Routing is
algebraically factorized so u_hat [B,I,K,D] (604 MB) is never materialized:

    y[s,k,j]  = sum_i c[s,i,k] x[s,i,j]       (per-sample PE matmul, i contracted)
    sT[d,k,s] = sum_j W[j,k,d] y[s,k,j]       (col-tiled quad PE matmuls, j contracted)
    v         = squash(sT)                    (ACT/DVE in transposed layout)
    t[j,k,s]  = sum_d W[j,k,d] vT[d,k,s]      (row-tiled quad PE matmuls, d contracted)
    b[s,i,k] += sum_j x[s,i,j] t[s,j,k]       (per-sample PE matmul, j contracted)

b is never materialized: c2 = softmax(b0+b1) uses exp(b0+b1) = exp(b0)*exp(b1)
(exp taken straight off the b-update PSUM by ACT). The squash sum-over-d runs
as a block-diagonal ones matmul on PE, which also replicates ss across each
32-partition group (free partition-broadcast). sqrt is computed as
exp(-0.5*ln(ss+eps)) so ACT stays inside the natural_log_exp table set the
whole kernel (no ACT_TABLE_LOAD switches). x is staged bf16 (i-major) for the
y-matmuls and fp8-e4m3 (j-major) for the agreement path; weights bf16.
"""
import numpy as np
import ml_dtypes
from contextlib import ExitStack

import concourse.bass as bass
import concourse.bacc as bacc_mod
import concourse.mybir as mybir
import concourse.tile as tile
from concourse.bass_utils import run_bass_kernel_spmd
from concourse.masks import make_identity

B, I, J, K, D = 128, 1152, 128, 32, 32
NCORES = 8
S = B // NCORES          # 16 samples per core
CH = I // 128            # 9 chunks of the input-capsule axis
NUM_ROUTING = 3
EPS = 1e-7
F32 = mybir.dt.float32
BF16 = mybir.dt.bfloat16
F8 = mybir.dt.float8e4
AF = mybir.ActivationFunctionType
SG = 4                   # softmax sample-group size

_PROGRAM = None
DEBUG = False
XB_FP8 = True


def _build_program():
    XB_DT = F8 if XB_FP8 else BF16
    nc = bacc_mod.Bacc("TRN2", target_bir_lowering=False, debug=False,
                       num_devices=NCORES)
    y0_d = nc.dram_tensor("y0", [128, S], BF16, kind="ExternalInput")
    wr_d = nc.dram_tensor("wr", [128, K, D], BF16, kind="ExternalInput")
    wt_d = nc.dram_tensor("wt", [128, K // 4, 128], BF16, kind="ExternalInput")
    xb_d = nc.dram_tensor("xb", [128, S, CH * 128], XB_DT, kind="ExternalInput")
    xa_d = nc.dram_tensor("xa", [128, S, CH, 128], BF16, kind="ExternalInput")
    # vout in [(kq, s), (g, d)] layout (k = 4*kq + g); unshuffled host-side
    v_d = nc.dram_tensor("vout", [128, 128], F32, kind="ExternalOutput")
    if DEBUG:
        dbg = {
            "d_sq0": nc.dram_tensor("d_sq0", [128, K // 4, S], BF16, kind="ExternalOutput"),
            "d_sT0": nc.dram_tensor("d_sT0", [128, K // 4, S], F32, kind="ExternalOutput"),
            "d_ss0": nc.dram_tensor("d_ss0", [128, K // 4, S], F32, kind="ExternalOutput"),
            "d_L0": nc.dram_tensor("d_L0", [128, K // 4, S], F32, kind="ExternalOutput"),
            "d_q0": nc.dram_tensor("d_q0", [128, K // 4, S], F32, kind="ExternalOutput"),
            "d_sc0": nc.dram_tensor("d_sc0", [128, K // 4, S], F32, kind="ExternalOutput"),
            "d_vT0": nc.dram_tensor("d_vT0", [128, K // 4, S], BF16, kind="ExternalOutput"),
            "d_T20": nc.dram_tensor("d_T20", [128, S, K], BF16, kind="ExternalOutput"),
            "d_exp0": nc.dram_tensor("d_exp0", [128, S, CH, K], BF16, kind="ExternalOutput"),
            "d_cs1": nc.dram_tensor("d_cs1", [128, S, CH, K], BF16, kind="ExternalOutput"),
            "d_Y21": nc.dram_tensor("d_Y21", [128, S, K], BF16, kind="ExternalOutput"),
            "d_exp1": nc.dram_tensor("d_exp1", [128, S, CH, K], BF16, kind="ExternalOutput"),
        }

    with tile.TileContext(nc) as tc, ExitStack() as ctx, \
            nc.allow_low_precision(reason="bf16 softmax stats validated offline"):
        const = ctx.enter_context(tc.tile_pool(name="const", bufs=1))
        xap = ctx.enter_context(tc.tile_pool(name="xa", bufs=1))
        xbp = ctx.enter_context(tc.tile_pool(name="xb", bufs=1))
        expp = ctx.enter_context(tc.tile_pool(name="exp", bufs=1))
        csp = ctx.enter_context(tc.tile_pool(name="cs", bufs=1))
        zp = ctx.enter_context(tc.tile_pool(name="z", bufs=2))
        sqp = ctx.enter_context(tc.tile_pool(name="sq", bufs=1))
        vtp = ctx.enter_context(tc.tile_pool(name="vt", bufs=1))
        t2p = ctx.enter_context(tc.tile_pool(name="t2", bufs=1))
        y2p = ctx.enter_context(tc.tile_pool(name="y2", bufs=1))
        outp = ctx.enter_context(tc.tile_pool(name="out", bufs=1))
        ps_sT = ctx.enter_context(tc.tile_pool(name="ps_sT", bufs=1, space="PSUM"))
        ps_t = ctx.enter_context(tc.tile_pool(name="ps_t", bufs=1, space="PSUM"))
        ps_y = ctx.enter_context(tc.tile_pool(name="ps_y", bufs=1, space="PSUM"))
        ps_bu = ctx.enter_context(tc.tile_pool(name="ps_bu", bufs=3, space="PSUM"))
        ps_m = ctx.enter_context(tc.tile_pool(name="ps_m", bufs=1, space="PSUM"))

        # ---- input DMAs. Two HWDGE queues (sync, scalar) so descriptor
        # generation parallelizes; FIFO order per queue = priority order.
        y0t = const.tile([128, S], BF16)
        nc.sync.dma_start(out=y0t, in_=y0_d[:])
        wr = const.tile([128, K, D], BF16)
        nc.sync.dma_start(out=wr, in_=wr_d[:])
        wt = const.tile([128, K // 4, 128], BF16)
        nc.scalar.dma_start(out=wt, in_=wt_d[:])
        xb = xbp.tile([128, S, CH * 128], XB_DT)
        xa = xap.tile([128, S, CH, 128], BF16)
        dma_q = [nc.sync, nc.scalar]
        for g in range(S // SG):
            dma_q[g % 2].dma_start(out=xb[:, g * SG:(g + 1) * SG],
                                   in_=xb_d[:, g * SG:(g + 1) * SG])
        for g in range(S // SG):
            dma_q[g % 2].dma_start(out=xa[:, g * SG:(g + 1) * SG],
                                   in_=xa_d[:, g * SG:(g + 1) * SG])

        eps128 = const.tile([128, 1], F32)
        nc.vector.memset(eps128, EPS)
        ident = const.tile([128, 128], F32)
        make_identity(nc, ident)

        # block-diagonal ones [128, 128]: ones2[p, m] = 1 iff p//32 == m//32
        ones2 = const.tile([128, 128], BF16)
        nc.vector.memset(ones2, 0.0)
        for g in range(4):
            nc.vector.memset(ones2[32 * g:32 * (g + 1), 32 * g:32 * (g + 1)], 1.0)
        # mask4[p, g] = 1 iff p//32 == g
        mask4 = const.tile([128, 4], BF16)
        nc.vector.memset(mask4, 0.0)
        for g in range(4):
            nc.vector.memset(mask4[32 * g:32 * (g + 1), g:g + 1], 1.0)

        exp0 = expp.tile([128, S, CH, K], BF16, tag="e0")
        exp1 = expp.tile([128, S, CH, K], BF16, tag="e1")

        # ACT table-set preloading: sqrt and exp live in different table
        # sets (~2.7us switch). Tiny dummy activations, data-chained to the
        # producing stage, force each switch to happen while PE streams run.
        dummy = const.tile([1, 1], F32)

        def preload(func, dep_ap):
            nc.scalar.activation(out=dummy, in_=dep_ap, func=func)

        preload(AF.Sqrt, eps128[0:1, :])  # load sqrt set during startup DMA

        def sT_matmuls(rhs_per_k):
            """sT_ps[(k%4)*32+d, k//4, s] = sum_j wr[j,k,d] * rhs_k[j,s]"""
            sT = ps_sT.tile([128, K // 4, S], F32, tag="sT")
            for kq in range(K // 4):
                for g in range(4):
                    k = 4 * kq + g
                    nc.tensor.matmul(sT[32 * g:32 * (g + 1), kq, :],
                                     wr[:, k, :], rhs_per_k(k),
                                     start=True, stop=True,
                                     tile_position=(0, 32 * g))
            return sT

        def squash01(sT, it, out_dt=BF16, do_preload=True):
            """vT[d,k,s] = squash(sT) along d, in the transposed layout.
            Square is a filler ACT func (no table load); Sqrt's table load
            is hidden by the preload dummies."""
            sq = sqp.tile([128, K // 4, S], BF16, tag="sq")
            nc.scalar.activation(out=sq, in_=sT, func=AF.Square)
            ss = ps_m.tile([128, K // 4, S], F32, tag="m")
            nc.tensor.matmul(ss, ones2, sq, start=True, stop=True)
            q = sqp.tile([128, K // 4, S], F32, tag="q")
            nc.scalar.activation(out=q, in_=ss, func=AF.Sqrt, bias=eps128)
            if do_preload:
                preload(AF.Exp, q[0:1, 0, 0:1])  # reload exp set during t/bu
            d1 = sqp.tile([128, K // 4, S], F32, tag="d1")
            nc.vector.scalar_tensor_tensor(out=d1, in0=ss, scalar=1.0, in1=q,
                                           op0=mybir.AluOpType.add,
                                           op1=mybir.AluOpType.mult)
            r1 = sqp.tile([128, K // 4, S], F32, tag="r1")
            nc.vector.reciprocal(out=r1, in_=d1)
            sc = sqp.tile([128, K // 4, S], F32, tag="sc")
            nc.vector.tensor_mul(sc, ss, r1)
            vT = vtp.tile([128, K // 4, S], out_dt, tag="vT" + str(out_dt))
            nc.vector.tensor_mul(vT, sT, sc)
            if DEBUG and it == 0:
                nc.scalar.dma_start(out=dbg["d_q0"][:], in_=q)
                nc.scalar.dma_start(out=dbg["d_sc0"][:], in_=sc)
            return vT

        def t_stage(vT, it):
            """T2[j, s, k] = sum_d W[j,k,d] vT[d,k,s].

            Per k-quad ONE matmul of N=64: rhs vTz[:, kq, g, s] is vT
            masked to partition group g, so contracting all 128 partitions
            picks out k = 4kq+g for output column (g, s)."""
            vTz = vtp.tile([128, K // 4, 4, S], BF16, tag="vTz")
            nc.vector.tensor_tensor(
                out=vTz,
                in0=vT.unsqueeze(2).broadcast_to([128, K // 4, 4, S]),
                in1=mask4.unsqueeze(1).unsqueeze(-1).broadcast_to([128, K // 4, 4, S]),
                op=mybir.AluOpType.mult)
            t_ps = ps_t.tile([128, K // 4, 4, S], F32, tag="t")
            for kq in range(K // 4):
                nc.tensor.matmul(t_ps[:, kq], wt[:, kq, :], vTz[:, kq],
                                 start=True, stop=True)
            T2 = t2p.tile([128, S, K], BF16, tag="T2")
            nc.vector.tensor_copy(
                out=T2.rearrange("p s (q g) -> p q g s", q=K // 4),
                in_=t_ps)
            return T2

        def bu_stage(T2, exp_t):
            """b-update for all samples; exp of each sample's bu -> exp_t."""
            for s in range(S):
                bu = ps_bu.tile([128, CH, K], F32, tag="bu")
                for ic in range(CH):
                    nc.tensor.matmul(bu[:, ic, :],
                                     xb[:, s, ic * 128:(ic + 1) * 128],
                                     T2[:, s, :], start=True, stop=True)
                nc.scalar.activation(out=exp_t[:, s], in_=bu, func=AF.Exp)
            preload(AF.Sqrt, exp_t[0:1, S - 1, 0, 0:1])  # sqrt set during y

        def y_matmuls(cs):
            y_ps = ps_y.tile([128, S, K], F32, tag="y")
            for s in range(S):
                for ic in range(CH):
                    nc.tensor.matmul(y_ps[:, s, :], xa[:, s, ic, :],
                                     cs[:, s, ic, :],
                                     start=(ic == 0), stop=(ic == CH - 1))
            Y2 = y2p.tile([128, S, K], BF16, tag="Y2")
            nc.scalar.activation(out=Y2, in_=y_ps, func=AF.Copy)
            return Y2

        # ================= iteration 0 =================
        sT = sT_matmuls(lambda k: y0t)
        vT = squash01(sT, 0)
        T2 = t_stage(vT, 0)
        bu_stage(T2, exp0)
        if DEBUG:
            nc.scalar.dma_start(out=dbg["d_vT0"][:], in_=vT)
            nc.scalar.dma_start(out=dbg["d_T20"][:], in_=T2)
            nc.scalar.dma_start(out=dbg["d_exp0"][:], in_=exp0)

        # ---- softmax it1: cs1 = exp0 / z
        cs = csp.tile([128, S, CH, K], BF16, tag="cs")
        for g in range(S // SG):
            sl = slice(g * SG, (g + 1) * SG)
            z = zp.tile([128, SG, CH], BF16, tag="z")
            nc.vector.tensor_reduce(out=z, in_=exp0[:, sl],
                                    axis=mybir.AxisListType.X,
                                    op=mybir.AluOpType.add)
            r = zp.tile([128, SG, CH], BF16, tag="r")
            nc.vector.reciprocal(out=r, in_=z)
            nc.vector.tensor_tensor(out=cs[:, sl], in0=exp0[:, sl],
                                    in1=r.unsqueeze(-1).broadcast_to([128, SG, CH, K]),
                                    op=mybir.AluOpType.mult)

        if DEBUG:
            nc.scalar.dma_start(out=dbg["d_cs1"][:], in_=cs)

        # ================= iteration 1 =================
        Y2 = y_matmuls(cs)
        sT = sT_matmuls(lambda k: Y2[:, :, k])
        vT = squash01(sT, 1)
        T2 = t_stage(vT, 1)
        bu_stage(T2, exp1)
        if DEBUG:
            nc.scalar.dma_start(out=dbg["d_Y21"][:], in_=Y2)
            nc.scalar.dma_start(out=dbg["d_exp1"][:], in_=exp1)

        # ---- softmax it2: cs2 = exp0*exp1 / z2
        cs2 = csp.tile([128, S, CH, K], BF16, tag="cs2")
        for g in range(S // SG):
            sl = slice(g * SG, (g + 1) * SG)
            e2 = csp.tile([128, SG, CH, K], BF16, tag="e2")
            nc.vector.tensor_tensor(out=e2, in0=exp0[:, sl], in1=exp1[:, sl],
                                    op=mybir.AluOpType.mult)
            z = zp.tile([128, SG, CH], BF16, tag="z")
            nc.vector.tensor_reduce(out=z, in_=e2, axis=mybir.AxisListType.X,
                                    op=mybir.AluOpType.add)
            r = zp.tile([128, SG, CH], BF16, tag="r")
            nc.vector.reciprocal(out=r, in_=z)
            nc.vector.tensor_tensor(out=cs2[:, sl], in0=e2,
                                    in1=r.unsqueeze(-1).broadcast_to([128, SG, CH, K]),
                                    op=mybir.AluOpType.mult)

        # ================= iteration 2 =================
        Y2 = y_matmuls(cs2)
        sT = sT_matmuls(lambda k: Y2[:, :, k])
        vT = squash01(sT, 2, out_dt=F32, do_preload=False)
        # transpose [(g,d), (kq,s)] -> [(kq,s), (g,d)] and store
        vt_ps = ps_m.tile([128, 128], F32, tag="m")
        nc.tensor.transpose(out=vt_ps, in_=vT.rearrange("p q s -> p (q s)"),
                            identity=ident)
        vsb = outp.tile([128, 128], F32, tag="v")
        nc.scalar.activation(out=vsb, in_=vt_ps, func=AF.Copy)
        nc.sync.dma_start(out=v_d[:], in_=vsb)

    nc.compile()
    return nc


def _get_program():
    global _PROGRAM
    if _PROGRAM is None:
        _PROGRAM = _build_program()
    return _PROGRAM


def _prep_core_inputs(x_core, Wf):
    """x_core: [S, I, J] fp32, Wf: [J, K, D] fp32 -> per-core input map."""
    bf = ml_dtypes.bfloat16
    f8 = ml_dtypes.float8_e4m3 if XB_FP8 else ml_dtypes.bfloat16
    y0 = np.ascontiguousarray((x_core.sum(axis=1, dtype=np.float32) / K).T.astype(bf))
    xa = np.ascontiguousarray(
        x_core.reshape(S, CH, 128, J).transpose(2, 0, 1, 3).astype(bf))
    xb = np.ascontiguousarray(
        x_core.transpose(2, 0, 1).astype(f8)).reshape(J, S, CH * 128)
    wr = np.ascontiguousarray(Wf.astype(bf))
    # wt[(k%4)*32 + d, k//4, j] = W[j, k, d]
    wt = np.ascontiguousarray(
        Wf.transpose(1, 2, 0).reshape(K // 4, 4, D, J)
        .transpose(1, 2, 0, 3).reshape(128, K // 4, J).astype(bf))
    return {"y0": y0, "wr": wr, "wt": wt, "xb": xb, "xa": xa}


def kernel(inputs, W):
    x = np.ascontiguousarray(np.asarray(inputs, dtype=np.float32))
    Wf = np.ascontiguousarray(np.asarray(W, dtype=np.float32))
    nc = _get_program()
    in_maps = [_prep_core_inputs(x[c * S:(c + 1) * S], Wf) for c in range(NCORES)]
    res = run_bass_kernel_spmd(nc, in_maps, list(range(NCORES)))
    return np.concatenate([_unshuffle_v(r["vout"]) for r in res.results], axis=0)


def _unshuffle_v(arr):
    """[(kq, s), (g, d)] fp32 -> [s, k, d] with k = 4*kq + g."""
    return np.ascontiguousarray(
        arr.reshape(K // 4, S, 4, D).transpose(1, 0, 2, 3).reshape(S, K, D))


# revision 61
# speedup vs baseline: 1.0370x; 1.0151x over previous
"""Trainium2 Bass kernel for CapsuleLayer dynamic routing (B=128, I=1152, J=128, K=32, D=32).

Strategy
--------
Data-parallel over batch: 16 samples per core x 8 cores. Routing is
algebraically factorized so u_hat [B,I,K,D] (604 MB) is never materialized:

    y[s,k,j]  = sum_i c[s,i,k] x[s,i,j]       (per-sample PE matmul, i contracted)
    sT[d,k,s] = sum_j W[j,k,d] y[s,k,j]       (col-tiled quad PE matmuls, j contracted)
    v         = squash(sT)                    (ACT/DVE in transposed layout)
    t[j,k,s]  = sum_d W[j,k,d] vT[d,k,s]      (row-tiled quad PE matmuls, d contracted)
    b[s,i,k] += sum_j x[s,i,j] t[s,j,k]       (per-sample PE matmul, j contracted)

b is never materialized: c2 = softmax(b0+b1) uses exp(b0+b1) = exp(b0)*exp(b1)
(exp taken straight off the b-update PSUM by ACT). The squash sum-over-d runs
as a block-diagonal ones matmul on PE, which also replicates ss across each
32-partition group (free partition-broadcast). sqrt is computed as
exp(-0.5*ln(ss+eps)) so ACT stays inside the natural_log_exp table set the
whole kernel (no ACT_TABLE_LOAD switches). x is staged bf16 (i-major) for the
y-matmuls and fp8-e4m3 (j-major) for the agreement path; weights bf16.
"""
import numpy as np
import ml_dtypes
from contextlib import ExitStack

import concourse.bass as bass
import concourse.bacc as bacc_mod
import concourse.mybir as mybir
import concourse.tile as tile
from concourse.bass_utils import run_bass_kernel_spmd
from concourse.masks import make_identity

B, I, J, K, D = 128, 1152, 128, 32, 32
NCORES = 8
S = B // NCORES          # 16 samples per core
CH = I // 128            # 9 chunks of the input-capsule axis
NUM_ROUTING = 3
EPS = 1e-7
F32 = mybir.dt.float32
BF16 = mybir.dt.bfloat16
F8 = mybir.dt.float8e4
AF = mybir.ActivationFunctionType
SG = 4                   # softmax sample-group size

_PROGRAM = None
DEBUG = False
XB_FP8 = True


def _build_program():
    XB_DT = F8 if XB_FP8 else BF16
    nc = bacc_mod.Bacc("TRN2", target_bir_lowering=False, debug=False,
                       num_devices=NCORES)
    y0_d = nc.dram_tensor("y0", [128, S], BF16, kind="ExternalInput")
    wr_d = nc.dram_tensor("wr", [128, K, D], BF16, kind="ExternalInput")
    wt_d = nc.dram_tensor("wt", [128, K // 4, 128], BF16, kind="ExternalInput")
    xb_d = nc.dram_tensor("xb", [128, S, CH * 128], XB_DT, kind="ExternalInput")
    xa_d = nc.dram_tensor("xa", [128, S, CH, 128], BF16, kind="ExternalInput")
    # vout in [(kq, s), (g, d)] layout (k = 4*kq + g); unshuffled host-side
    v_d = nc.dram_tensor("vout", [128, 128], F32, kind="ExternalOutput")
    if DEBUG:
        dbg = {
            "d_sq0": nc.dram_tensor("d_sq0", [128, K // 4, S], BF16, kind="ExternalOutput"),
            "d_sT0": nc.dram_tensor("d_sT0", [128, K // 4, S], F32, kind="ExternalOutput"),
            "d_ss0": nc.dram_tensor("d_ss0", [128, K // 4, S], F32, kind="ExternalOutput"),
            "d_L0": nc.dram_tensor("d_L0", [128, K // 4, S], F32, kind="ExternalOutput"),
            "d_q0": nc.dram_tensor("d_q0", [128, K // 4, S], F32, kind="ExternalOutput"),
            "d_sc0": nc.dram_tensor("d_sc0", [128, K // 4, S], F32, kind="ExternalOutput"),
            "d_vT0": nc.dram_tensor("d_vT0", [128, K // 4, S], BF16, kind="ExternalOutput"),
            "d_T20": nc.dram_tensor("d_T20", [128, S, K], BF16, kind="ExternalOutput"),
            "d_exp0": nc.dram_tensor("d_exp0", [128, S, CH, K], BF16, kind="ExternalOutput"),
            "d_cs1": nc.dram_tensor("d_cs1", [128, S, CH, K], BF16, kind="ExternalOutput"),
            "d_Y21": nc.dram_tensor("d_Y21", [128, S, K], BF16, kind="ExternalOutput"),
            "d_exp1": nc.dram_tensor("d_exp1", [128, S, CH, K], BF16, kind="ExternalOutput"),
        }

    with tile.TileContext(nc) as tc, ExitStack() as ctx, \
            nc.allow_low_precision(reason="bf16 softmax stats validated offline"):
        const = ctx.enter_context(tc.tile_pool(name="const", bufs=1))
        xap = ctx.enter_context(tc.tile_pool(name="xa", bufs=1))
        xbp = ctx.enter_context(tc.tile_pool(name="xb", bufs=1))
        expp = ctx.enter_context(tc.tile_pool(name="exp", bufs=1))
        csp = ctx.enter_context(tc.tile_pool(name="cs", bufs=1))
        zp = ctx.enter_context(tc.tile_pool(name="z", bufs=2))
        sqp = ctx.enter_context(tc.tile_pool(name="sq", bufs=1))
        vtp = ctx.enter_context(tc.tile_pool(name="vt", bufs=1))
        t2p = ctx.enter_context(tc.tile_pool(name="t2", bufs=1))
        y2p = ctx.enter_context(tc.tile_pool(name="y2", bufs=1))
        outp = ctx.enter_context(tc.tile_pool(name="out", bufs=1))
        ps_sT = ctx.enter_context(tc.tile_pool(name="ps_sT", bufs=1, space="PSUM"))
        ps_t = ctx.enter_context(tc.tile_pool(name="ps_t", bufs=1, space="PSUM"))
        ps_y = ctx.enter_context(tc.tile_pool(name="ps_y", bufs=1, space="PSUM"))
        ps_bu = ctx.enter_context(tc.tile_pool(name="ps_bu", bufs=3, space="PSUM"))
        ps_m = ctx.enter_context(tc.tile_pool(name="ps_m", bufs=1, space="PSUM"))
        ps_w = ctx.enter_context(tc.tile_pool(name="ps_w", bufs=1, space="PSUM"))

        # ---- input DMAs. Two HWDGE queues (sync, scalar) so descriptor
        # generation parallelizes; FIFO order per queue = priority order.
        y0t = const.tile([128, S], BF16)
        nc.sync.dma_start(out=y0t, in_=y0_d[:])
        wr = const.tile([128, K, D], BF16)
        nc.sync.dma_start(out=wr, in_=wr_d[:])
        wt = const.tile([128, K // 4, 128], BF16)
        nc.scalar.dma_start(out=wt, in_=wt_d[:])
        xb = xbp.tile([128, S, CH * 128], XB_DT)
        xa = xap.tile([128, S, CH, 128], BF16)
        dma_q = [nc.sync, nc.scalar]
        for g in range(S // SG):
            dma_q[g % 2].dma_start(out=xb[:, g * SG:(g + 1) * SG],
                                   in_=xb_d[:, g * SG:(g + 1) * SG])
        for g in range(S // SG):
            dma_q[g % 2].dma_start(out=xa[:, g * SG:(g + 1) * SG],
                                   in_=xa_d[:, g * SG:(g + 1) * SG])

        eps128 = const.tile([128, 1], F32)
        nc.vector.memset(eps128, EPS)
        ident = const.tile([128, 128], F32)
        make_identity(nc, ident)

        # block-diagonal ones [128, 128]: ones2[p, m] = 1 iff p//32 == m//32
        ones2 = const.tile([128, 128], BF16)
        nc.vector.memset(ones2, 0.0)
        for g in range(4):
            nc.vector.memset(ones2[32 * g:32 * (g + 1), 32 * g:32 * (g + 1)], 1.0)
        # mask4[p, g] = 1 iff p//32 == g
        mask4 = const.tile([128, 4], BF16)
        nc.vector.memset(mask4, 0.0)
        for g in range(4):
            nc.vector.memset(mask4[32 * g:32 * (g + 1), g:g + 1], 1.0)

        exp0 = expp.tile([128, S, CH, K], BF16, tag="e0")
        exp1 = expp.tile([128, S, CH, K], BF16, tag="e1")

        # ACT table-set preloading: sqrt and exp live in different table
        # sets (~2.7us switch). Tiny dummy activations, data-chained to the
        # producing stage, force each switch to happen while PE streams run.
        dummy = const.tile([1, 1], F32)

        def preload(func, dep_ap):
            nc.scalar.activation(out=dummy, in_=dep_ap, func=func)

        preload(AF.Sqrt, eps128[0:1, :])  # load sqrt set during startup DMA

        def warm_mm(dep_f32):
            """Tiny PE op chained to a mid-squash value: keeps the HAM
            activity window busy so the PE clock stays at 2.4 GHz through
            the squash/softmax gaps."""
            w = ps_w.tile([1, 1], F32, tag="w")
            nc.tensor.matmul(w, ident[0:1, 0:1], dep_f32, start=True, stop=True)

        def sT_matmuls(rhs_per_k):
            """sT_ps[(k%4)*32+d, k//4, s] = sum_j wr[j,k,d] * rhs_k[j,s]"""
            sT = ps_sT.tile([128, K // 4, S], F32, tag="sT")
            for kq in range(K // 4):
                for g in range(4):
                    k = 4 * kq + g
                    nc.tensor.matmul(sT[32 * g:32 * (g + 1), kq, :],
                                     wr[:, k, :], rhs_per_k(k),
                                     start=True, stop=True,
                                     tile_position=(0, 32 * g))
            return sT

        def squash01(sT, it, out_dt=BF16, do_preload=True):
            """vT[d,k,s] = squash(sT) along d, in the transposed layout.
            Square is a filler ACT func (no table load); Sqrt's table load
            is hidden by the preload dummies."""
            sq = sqp.tile([128, K // 4, S], BF16, tag="sq")
            nc.scalar.activation(out=sq, in_=sT, func=AF.Square)
            ss = ps_m.tile([128, K // 4, S], F32, tag="m")
            nc.tensor.matmul(ss, ones2, sq, start=True, stop=True)
            q = sqp.tile([128, K // 4, S], F32, tag="q")
            nc.scalar.activation(out=q, in_=ss, func=AF.Sqrt, bias=eps128)
            if do_preload:
                preload(AF.Exp, q[0:1, 0, 0:1])  # reload exp set during t/bu
            warm_mm(q[0:1, 0, 0:1])
            d1 = sqp.tile([128, K // 4, S], F32, tag="d1")
            nc.vector.scalar_tensor_tensor(out=d1, in0=ss, scalar=1.0, in1=q,
                                           op0=mybir.AluOpType.add,
                                           op1=mybir.AluOpType.mult)
            r1 = sqp.tile([128, K // 4, S], F32, tag="r1")
            nc.vector.reciprocal(out=r1, in_=d1)
            warm_mm(r1[0:1, 0, 0:1])
            sc = sqp.tile([128, K // 4, S], F32, tag="sc")
            nc.vector.tensor_mul(sc, ss, r1)
            warm_mm(sc[0:1, 0, 0:1])
            vT = vtp.tile([128, K // 4, S], out_dt, tag="vT" + str(out_dt))
            nc.vector.tensor_mul(vT, sT, sc)
            if DEBUG and it == 0:
                nc.scalar.dma_start(out=dbg["d_q0"][:], in_=q)
                nc.scalar.dma_start(out=dbg["d_sc0"][:], in_=sc)
            return vT

        def t_stage(vT, it):
            """T2[j, s, k] = sum_d W[j,k,d] vT[d,k,s].

            Per k-quad ONE matmul of N=64: rhs vTz[:, kq, g, s] is vT
            masked to partition group g, so contracting all 128 partitions
            picks out k = 4kq+g for output column (g, s)."""
            vTz = vtp.tile([128, K // 4, 4, S], BF16, tag="vTz")
            nc.vector.tensor_tensor(
                out=vTz,
                in0=vT.unsqueeze(2).broadcast_to([128, K // 4, 4, S]),
                in1=mask4.unsqueeze(1).unsqueeze(-1).broadcast_to([128, K // 4, 4, S]),
                op=mybir.AluOpType.mult)
            t_ps = ps_t.tile([128, K // 4, 4, S], F32, tag="t")
            for kq in range(K // 4):
                nc.tensor.matmul(t_ps[:, kq], wt[:, kq, :], vTz[:, kq],
                                 start=True, stop=True)
            T2 = t2p.tile([128, S, K], BF16, tag="T2")
            nc.vector.tensor_copy(
                out=T2.rearrange("p s (q g) -> p q g s", q=K // 4),
                in_=t_ps)
            return T2

        def bu_stage(T2, exp_t):
            """b-update for all samples; exp of each sample's bu -> exp_t."""
            for s in range(S):
                bu = ps_bu.tile([128, CH, K], F32, tag="bu")
                for ic in range(CH):
                    nc.tensor.matmul(bu[:, ic, :],
                                     xb[:, s, ic * 128:(ic + 1) * 128],
                                     T2[:, s, :], start=True, stop=True)
                nc.scalar.activation(out=exp_t[:, s], in_=bu, func=AF.Exp)
            preload(AF.Sqrt, exp_t[0:1, S - 1, 0, 0:1])  # sqrt set during y

        def y_matmuls(cs):
            y_ps = ps_y.tile([128, S, K], F32, tag="y")
            for s in range(S):
                for ic in range(CH):
                    nc.tensor.matmul(y_ps[:, s, :], xa[:, s, ic, :],
                                     cs[:, s, ic, :],
                                     start=(ic == 0), stop=(ic == CH - 1))
            Y2 = y2p.tile([128, S, K], BF16, tag="Y2")
            nc.vector.tensor_copy(out=Y2, in_=y_ps)
            return Y2

        # ================= iteration 0 =================
        sT = sT_matmuls(lambda k: y0t)
        vT = squash01(sT, 0)
        T2 = t_stage(vT, 0)
        bu_stage(T2, exp0)
        if DEBUG:
            nc.scalar.dma_start(out=dbg["d_vT0"][:], in_=vT)
            nc.scalar.dma_start(out=dbg["d_T20"][:], in_=T2)
            nc.scalar.dma_start(out=dbg["d_exp0"][:], in_=exp0)

        # ---- softmax it1: cs1 = exp0 / z
        cs = csp.tile([128, S, CH, K], BF16, tag="cs")
        for g in range(S // SG):
            sl = slice(g * SG, (g + 1) * SG)
            z = zp.tile([128, SG, CH], BF16, tag="z")
            nc.vector.tensor_reduce(out=z, in_=exp0[:, sl],
                                    axis=mybir.AxisListType.X,
                                    op=mybir.AluOpType.add)
            r = zp.tile([128, SG, CH], BF16, tag="r")
            nc.vector.reciprocal(out=r, in_=z)
            nc.vector.tensor_tensor(out=cs[:, sl], in0=exp0[:, sl],
                                    in1=r.unsqueeze(-1).broadcast_to([128, SG, CH, K]),
                                    op=mybir.AluOpType.mult)

        if DEBUG:
            nc.scalar.dma_start(out=dbg["d_cs1"][:], in_=cs)

        # ================= iteration 1 =================
        Y2 = y_matmuls(cs)
        sT = sT_matmuls(lambda k: Y2[:, :, k])
        vT = squash01(sT, 1)
        T2 = t_stage(vT, 1)
        bu_stage(T2, exp1)
        if DEBUG:
            nc.scalar.dma_start(out=dbg["d_Y21"][:], in_=Y2)
            nc.scalar.dma_start(out=dbg["d_exp1"][:], in_=exp1)

        # ---- softmax it2: cs2 = exp0*exp1 / z2
        cs2 = csp.tile([128, S, CH, K], BF16, tag="cs2")
        for g in range(S // SG):
            sl = slice(g * SG, (g + 1) * SG)
            e2 = csp.tile([128, SG, CH, K], BF16, tag="e2")
            nc.vector.tensor_tensor(out=e2, in0=exp0[:, sl], in1=exp1[:, sl],
                                    op=mybir.AluOpType.mult)
            z = zp.tile([128, SG, CH], BF16, tag="z")
            nc.vector.tensor_reduce(out=z, in_=e2, axis=mybir.AxisListType.X,
                                    op=mybir.AluOpType.add)
            r = zp.tile([128, SG, CH], BF16, tag="r")
            nc.vector.reciprocal(out=r, in_=z)
            nc.vector.tensor_tensor(out=cs2[:, sl], in0=e2,
                                    in1=r.unsqueeze(-1).broadcast_to([128, SG, CH, K]),
                                    op=mybir.AluOpType.mult)

        # ================= iteration 2 =================
        Y2 = y_matmuls(cs2)
        sT = sT_matmuls(lambda k: Y2[:, :, k])
        vT = squash01(sT, 2, out_dt=F32, do_preload=False)
        # transpose [(g,d), (kq,s)] -> [(kq,s), (g,d)] and store
        vt_ps = ps_m.tile([128, 128], F32, tag="m")
        nc.tensor.transpose(out=vt_ps, in_=vT.rearrange("p q s -> p (q s)"),
                            identity=ident)
        vsb = outp.tile([128, 128], F32, tag="v")
        nc.scalar.activation(out=vsb, in_=vt_ps, func=AF.Copy)
        nc.sync.dma_start(out=v_d[:], in_=vsb)

    nc.compile()
    return nc


def _get_program():
    global _PROGRAM
    if _PROGRAM is None:
        _PROGRAM = _build_program()
    return _PROGRAM


def _prep_core_inputs(x_core, Wf):
    """x_core: [S, I, J] fp32, Wf: [J, K, D] fp32 -> per-core input map."""
    bf = ml_dtypes.bfloat16
    f8 = ml_dtypes.float8_e4m3 if XB_FP8 else ml_dtypes.bfloat16
    y0 = np.ascontiguousarray((x_core.sum(axis=1, dtype=np.float32) / K).T.astype(bf))
    xa = np.ascontiguousarray(
        x_core.reshape(S, CH, 128, J).transpose(2, 0, 1, 3).astype(bf))
    xb = np.ascontiguousarray(
        x_core.transpose(2, 0, 1).astype(f8)).reshape(J, S, CH * 128)
    wr = np.ascontiguousarray(Wf.astype(bf))
    # wt[(k%4)*32 + d, k//4, j] = W[j, k, d]
    wt = np.ascontiguousarray(
        Wf.transpose(1, 2, 0).reshape(K // 4, 4, D, J)
        .transpose(1, 2, 0, 3).reshape(128, K // 4, J).astype(bf))
    return {"y0": y0, "wr": wr, "wt": wt, "xb": xb, "xa": xa}


def kernel(inputs, W):
    x = np.ascontiguousarray(np.asarray(inputs, dtype=np.float32))
    Wf = np.ascontiguousarray(np.asarray(W, dtype=np.float32))
    nc = _get_program()
    in_maps = [_prep_core_inputs(x[c * S:(c + 1) * S], Wf) for c in range(NCORES)]
    res = run_bass_kernel_spmd(nc, in_maps, list(range(NCORES)))
    return np.concatenate([_unshuffle_v(r["vout"]) for r in res.results], axis=0)


def _unshuffle_v(arr):
    """[(kq, s), (g, d)] fp32 -> [s, k, d] with k = 4*kq + g."""
    return np.ascontiguousarray(
        arr.reshape(K // 4, S, 4, D).transpose(1, 0, 2, 3).reshape(S, K, D))
